# revision 1
# baseline (speedup 1.0000x reference)
"""AttentiveConv3d (sparse_attention) Trainium2 kernel — self-contained.

kernel(**inputs) takes the FULL inputs
    x     [2, 128, 16, 28, 28] f32
    q     [2, 1, 64] f32
    W_out [128, 128] f32
    b_out [128] f32
and returns the FULL output [2, 128, 16, 28, 28] f32.

Sharding: data-parallel over (batch, T-chunks): 8 cores, core i handles
batch i//4, output frames 4*(i%4) .. 4*(i%4)+3, with a 1-frame halo supplied
by host-side padding/slicing (no device collectives needed).

Math (equivalent to the reference; softmax computed without max-subtraction,
valid because |logits| < ~0.2 for this operator's scaling):
    z   = qmask^T @ xp        (per padded location; both heads)
    E   = exp(z);  F = E * xp
    num = Box3x3x3(F); d = Box3x3x3(E)    (separable box filters)
    y   = W_out @ (num / d) + b_out

Engine plan per core:
    PE   : z matmul, T-pass of F (identity accumulate), Box3d+head-select of
           the denominator pack (9 shifted-rhs accumulating matmuls),
           1/d broadcast, output projection  (all float32r)
    ACT  : exp, middle-tap copy for the W-pass, bias PSUM evacuation
    DVE  : W-pass of F, reciprocal, merged = num*r, some F and H ops
    Pool : most F = E*xp multiplies and H-pass shifted adds
"""
from contextlib import ExitStack

import numpy as np

import concourse.bass as bass
import concourse.tile as tile
from concourse import bacc, mybir
from concourse import bass_utils
from concourse.dve_ops import RECIP_APPROX_FAST_CONSTS, RECIPROCAL_APPROX_FAST

F32 = mybir.dt.float32
F32R = mybir.dt.float32r
AF = mybir.ActivationFunctionType

C = 128
TIN, TOUT = 6, 4
HP, WP = 30, 30
HO, WO = 28, 28
NF = HP * WP        # 900
NOF = HO * WO       # 784
BLOB_W = 390        # qm 0:128 | idm 128:256 | wt 256:384 | bias 384:385 (f32 bits)


def _build_nc(num_devices=8, reps=1, f_dve_frames=4, n_warm=10, h_pool_frames=(0, 1, 3), w_pool_frames=()):
    nc = bacc.Bacc("TRN2", target_bir_lowering=False, debug=False,
                   num_devices=num_devices)
    d_xp = nc.dram_tensor("xp", [C, TIN, HO, WO], F32R, kind="ExternalInput").ap()
    d_blob = nc.dram_tensor("blob", [C, BLOB_W], F32R, kind="ExternalInput").ap()
    d_sel = nc.dram_tensor("sel", [36, 16 + TOUT * C], F32R, kind="ExternalInput").ap()
    d_y = nc.dram_tensor("y", [C, TOUT, HO, WO], F32, kind="ExternalOutput").ap()

    with tile.TileContext(nc) as tc:
        with ExitStack() as ctx:
            consts = ctx.enter_context(tc.tile_pool(name="consts", bufs=1))
            sb_x = ctx.enter_context(tc.tile_pool(name="sb_x", bufs=1))
            sb_e = ctx.enter_context(tc.tile_pool(name="sb_e", bufs=1))
            sb_f = ctx.enter_context(tc.tile_pool(name="sb_f", bufs=1))
            sb_g = ctx.enter_context(tc.tile_pool(name="sb_g", bufs=3))
            sb_tmp = ctx.enter_context(tc.tile_pool(name="sb_tmp", bufs=3))
            sb_m = ctx.enter_context(tc.tile_pool(name="sb_m", bufs=3))
            sb_y = ctx.enter_context(tc.tile_pool(name="sb_y", bufs=3))
            sb_ep = ctx.enter_context(tc.tile_pool(name="sb_ep", bufs=1))
            ps_big = ctx.enter_context(tc.tile_pool(name="ps_big", bufs=3, space="PSUM"))
            ps_small = ctx.enter_context(tc.tile_pool(name="ps_small", bufs=2, space="PSUM"))

            blob_t = consts.tile([C, BLOB_W], F32R)
            nc.sync.dma_start(out=blob_t[:], in_=d_blob[:])
            sel_t = consts.tile([36, 16 + TOUT * C], F32R)
            nc.scalar.dma_start(out=sel_t[:], in_=d_sel[:])
            qm = blob_t[:, 0:128]
            idm = blob_t[:, 128:256]
            wt = blob_t[:, 256:384]
            bias = blob_t[:, 384:385].bitcast(F32)
            sel36a = sel_t[:, 0:8]       # rows 0:12 (dy=0) T+head select
            selr = sel_t[0:8, 16:].rearrange("p (t c) -> p t c", t=TOUT)

            for i in range(n_warm):
                wrm = ps_small.tile([C, 384], F32, tag="small", name=f"warm{i}")
                nc.tensor.matmul(wrm[:], qm, blob_t[:, 0:384], start=True, stop=True)

            for _ in range(reps):
                _body(tc, nc, d_xp, d_y, qm, idm, wt, bias, sel36a, selr,
                      sb_x, sb_e, sb_f, sb_g, sb_tmp, sb_m, sb_y, sb_ep,
                      ps_big, ps_small, f_dve_frames, h_pool_frames, w_pool_frames)
    nc.compile()
    return nc


def _body(tc, nc, d_xp, d_y, qm, idm, wt, bias, sel36a, selr,
          sb_x, sb_e, sb_f, sb_g, sb_tmp, sb_m, sb_y, sb_ep,
          ps_big, ps_small, f_dve_frames, h_pool_frames, w_pool_frames):
    # ---- phase A: load interior, synthesize pads, z, E = exp(z), F = E*xp
    # Input ships unpadded ([C, 6, 28, 28] contiguous). Pad positions of the
    # E field are exp(0) = 1 and of the F field are 0; both are written once
    # by GpSimd memsets during the (otherwise idle) DMA-load window.
    e128 = sb_e.tile([C, TIN * NF], F32R, tag="e128")
    ebase = e128[:]
    # E pads = 1.0: y-border rows of every frame, then x-border cols
    erows = bass.AP(tensor=ebase.tensor, offset=ebase.offset,
                    ap=[list(ebase.ap[0]), [NF, TIN], [870, 2], [1, WP]])
    # ACT Copy with scale=0, bias=1 writes the constant 1.0 through any AP
    nc.scalar.activation(erows, erows, AF.Copy, bias=1.0, scale=0.0)
    ecols = bass.AP(tensor=ebase.tensor, offset=ebase.offset + WP,
                    ap=[list(ebase.ap[0]), [NF, TIN], [WP, HP - 2], [WP - 1, 2]])
    nc.scalar.activation(ecols, ecols, AF.Copy, bias=1.0, scale=0.0)

    f_tiles = []
    for f in range(TIN):
        xtt = sb_x.tile([C, NOF], F32R, tag=f"x{f}", name=f"xt{f}")
        nc.sync.dma_start(out=xtt[:], in_=d_xp[:, f])
        xt = xtt[:]

        ft = sb_f.tile([C, NF], F32R, tag=f"f{f}", name=f"ft{f}")
        fbase = ft[:]
        frows = bass.AP(tensor=fbase.tensor, offset=fbase.offset,
                        ap=[list(fbase.ap[0]), [870, 2], [1, WP]])
        nc.scalar.activation(frows, frows, AF.Copy, bias=0.0, scale=0.0)
        fcols = bass.AP(tensor=fbase.tensor, offset=fbase.offset + WP,
                        ap=[list(fbase.ap[0]), [WP, HP - 2], [WP - 1, 2]])
        nc.scalar.activation(fcols, fcols, AF.Copy, bias=0.0, scale=0.0)

        zp = ps_big.tile([C, 1024], F32, tag="big", name=f"zp{f}")
        nc.tensor.matmul(zp[:, 0:392], qm, xt[:, 0:392], start=True, stop=True)
        nc.tensor.matmul(zp[:, 512:904], qm, xt[:, 392:784], start=True, stop=True)

        zv = zp[:].rearrange("p (b k) -> p b k", b=2)[:, :, 0:392]
        e_frame = e128[:, f * NF:(f + 1) * NF].rearrange("p (y x) -> p y x", y=HP)
        e_int = e_frame[:, 1:29, 1:29]
        ev = e_int.rearrange("p (c y) x -> p c y x", c=2)
        nc.scalar.activation(ev, zv, AF.Exp)

        f_int = ft[:].rearrange("p (y x) -> p y x", y=HP)[:, 1:29, 1:29]
        xv = xt.rearrange("p (y x) -> p y x", y=HO)
        if f >= TIN - f_dve_frames:
            nc.vector.tensor_mul(f_int, e_int, xv)
        else:
            nc.gpsimd.tensor_mul(f_int, e_int, xv)
        f_tiles.append(ft)

    def _num_part(t):
        # T-pass of F on PE: ft = F[t] + F[t+1] + F[t+2] (identity accumulate)
        ftp = ps_big.tile([C, 1024], F32, tag="big", name=f"ftp{t}")
        for half in range(2):
            lo, hi = half * 512, half * 512 + 450
            slo = half * 450
            for dt in range(3):
                nc.tensor.matmul(ftp[:, lo:hi], idm,
                                 f_tiles[t + dt][:, slo:slo + 450],
                                 start=(dt == 0), stop=(dt == 2))
        # W-pass on DVE; middle tap via ACT copy (single-PSUM-operand rule)
        ftv = (ftp[:].rearrange("p (b k) -> p b k", b=2)[:, :, 0:450]
               .rearrange("p b (r x) -> p b r x", x=WP))
        cpw = sb_tmp.tile([C, HP, WO], F32, tag="cpw", name=f"cpw{t}")
        cpv = cpw[:].rearrange("p (b r) x -> p b r x", b=2)
        nc.scalar.copy(cpv, ftv[:, :, :, 1:29])
        gt = sb_g.tile([C, HP, WO], F32R, tag="g", name=f"gt{t}")
        gv = gt[:].rearrange("p (b r) x -> p b r x", b=2)
        wtmp = sb_tmp.tile([C, HP, WO], F32, tag="wtmp", name=f"wtmp{t}")
        wv = wtmp[:].rearrange("p (b r) x -> p b r x", b=2)
        if t in w_pool_frames:
            # second ACT copy makes the first W-add all-SBUF -> run it on Pool
            cp0 = sb_tmp.tile([C, HP, WO], F32, tag="cp0", name=f"cp0{t}")
            c0v = cp0[:].rearrange("p (b r) x -> p b r x", b=2)
            nc.scalar.copy(c0v, ftv[:, :, :, 0:28])
            nc.gpsimd.tensor_add(wv, cpv, c0v)
        else:
            nc.vector.tensor_add(wv, cpv, ftv[:, :, :, 0:28])
        nc.vector.tensor_add(gv, wv, ftv[:, :, :, 2:30])
        return gt

    def _tail_part(t, gt, r8):
        mt = sb_m.tile([C, NOF], F32R, tag="m", name=f"mt{t}")
        # H-pass on Pool (gt is SBUF); merged reads r-broadcast PSUM directly
        htmp = sb_tmp.tile([C, HO, WO], F32, tag="htmp", name=f"htmp{t}")
        numt = sb_tmp.tile([C, HO, WO], F32, tag="numt", name=f"numt{t}", bufs=2)
        heng = nc.gpsimd if t in h_pool_frames else nc.vector
        heng.tensor_add(htmp[:], gt[:, 0:28, :], gt[:, 1:29, :])
        heng.tensor_add(numt[:], htmp[:], gt[:, 2:30, :])
        nv = numt[:].rearrange("p y x -> p (y x)")
        for ch in range(2):
            rp = ps_small.tile([C, 392], F32, tag="small", name=f"rp{t}_{ch}")
            nc.tensor.matmul(rp[:], selr[:, t, :], r8[:, ch * 392:ch * 392 + 392],
                             start=True, stop=True)
            nc.vector.tensor_mul(mt[:, ch * 392:ch * 392 + 392],
                                 nv[:, ch * 392:ch * 392 + 392], rp[:])

        yt = sb_y.tile([C, NOF], F32, tag="y", name=f"yt{t}")
        for ch in range(2):
            yp = ps_small.tile([C, 392], F32, tag="small", name=f"yp{t}_{ch}")
            nc.tensor.matmul(yp[:], wt, mt[:, ch * 392:ch * 392 + 392],
                             start=True, stop=True)
            nc.scalar.activation(yt[:, ch * 392:ch * 392 + 392], yp[:],
                                 AF.Identity, bias=bias, scale=1.0)
        nc.sync.dma_start(out=d_y[:, t], in_=yt[:])

    g0 = _num_part(0)
    g1 = _num_part(1)
    # ---- phase B: denominator path -----------------------------------
    # pack12 [12, 900]: row = 6h + t holds E_h[t] as (y, x). One PE matmul
    # group per chunk contracts rows (T-pass + head select) while the rhs
    # access pattern supplies the 9 (dy, dx) shifts -> full Box3d + select
    # with zero extra DVE work and zero repack DMAs.
    ep1 = sb_ep.tile([12, NF], F32R, tag="ep1")
    for h in range(2):
        nc.sync.dma_start(
            out=ep1[6 * h:6 * h + 6, :],
            in_=e128[h:h + 1, :].rearrange("p (t q) -> p t q", t=TIN))
    epv = ep1[:].rearrange("p (y x) -> p y x", y=HP)
    r8f = sb_ep.tile([8, NOF], F32, tag="r8f")
    for ch in range(2):
        ylo = 14 * ch
        d8p = ps_small.tile([8, 392], F32, tag="small", name=f"d8p{ch}")
        k = 0
        for dy in range(3):
            for dx in range(3):
                nc.tensor.matmul(
                    d8p[:], sel36a[0:12, :],
                    epv[:, ylo + dy:ylo + dy + 14, dx:dx + WO],
                    start=(k == 0), stop=(k == 8))
                k += 1
        nc.vector.reciprocal_approx_fast(r8f[:, ch * 392:ch * 392 + 392], d8p[:])
    r8 = sb_ep.tile([8, NOF], F32R, tag="r8")
    nc.scalar.copy(r8[:], r8f[:])
    _tail_part(0, g0, r8)
    _tail_part(1, g1, r8)
    g2 = _num_part(2)
    _tail_part(2, g2, r8)
    g3 = _num_part(3)
    _tail_part(3, g3, r8)


# ---------------------------------------------------------------------------
# Host side
# ---------------------------------------------------------------------------

def _host_prep(x, q, W_out, b_out):
    B, C_, T, H, W = x.shape
    heads, hs = 2, 64
    xpad = np.zeros((B, C_, T + 2, HP, WP), np.float32)
    xpad[:, :, 1:T + 1, 1:H + 1, 1:W + 1] = x

    cidx = np.arange(C_)
    qfull = (np.asarray(q, np.float32)[cidx % heads, 0, cidx // heads] / hs)
    qm = np.zeros((C_, C_), np.float32)
    for m in range(C_):
        qm[:, m] = np.where(cidx % heads == m % heads, qfull, 0.0)
    idm = np.eye(C_, dtype=np.float32)
    wt = np.ascontiguousarray(np.asarray(W_out, np.float32).T)
    bias = np.asarray(b_out, np.float32).reshape(C_, 1)
    blob = np.zeros((C_, BLOB_W), np.float32)
    blob[:, 0:128] = qm
    blob[:, 128:256] = idm
    blob[:, 256:384] = wt
    blob[:, 384:385] = bias

    # sel tensor: [36, 16 + TOUT*C]
    sel = np.zeros((36, 16 + TOUT * C_), np.float32)
    for h in range(2):
        for t in range(TIN):
            for tp in range(TOUT):
                if 0 <= t - tp <= 2:
                    sel[6 * h + t, 4 * h + tp] = 1.0              # dy=0 block
                    for dy in (1, 2):
                        sel[12 * (dy - 1) + 6 * h + t, 8 + 4 * h + tp] = 1.0
    selr = np.zeros((8, TOUT, C_), np.float32)
    for tp in range(TOUT):
        selr[4 * (cidx % heads) + tp, tp, cidx] = 1.0
    sel[0:8, 16:] = selr.reshape(8, TOUT * C_)
    shared = {"blob": blob, "sel": sel}
    in_maps = []
    for core in range(8):
        b, t0 = core // 4, (core % 4) * 4
        xp = np.ascontiguousarray(xpad[b, :, t0:t0 + TIN, 1:29, 1:29])
        in_maps.append({"xp": xp, **shared})
    return in_maps


_NC_CACHE = {}


def _get_nc(reps=1):
    if reps not in _NC_CACHE:
        _NC_CACHE[reps] = _build_nc(reps=reps)
    return _NC_CACHE[reps]


def kernel(x, q, W_out, b_out):
    x = np.asarray(x, np.float32)
    in_maps = _host_prep(x, q, W_out, b_out)
    nc = _get_nc()
    res = bass_utils.run_bass_kernel_spmd(nc, in_maps, list(range(8)))
    y = np.zeros((2, 128, 16, 28, 28), np.float32)
    for core in range(8):
        b, t0 = core // 4, (core % 4) * 4
        y[b, :, t0:t0 + TOUT] = res.results[core]["y"]
    return y

